# revision 34
# baseline (speedup 1.0000x reference)
"""Trainium2 Bass kernel for nn_MultiMPNN (gnn_message_passing).

Reference computation (B=4, N=512, Z=64, E=16, H=128):
    msgs[b,i,j,:] = z[b,i]@W_i + z[b,j]@W_j + e_feat[b,i,j]@W_e + b_msg
    agg[b,i,:]    = max_j (msgs + (adj>0 ? 0 : -inf))
    out           = z@Wu_z + agg@Wu_h + b_upd

Sharding: 8 cores = (batch b, half of destination rows i).  Each core owns
256 i-rows and the full j axis.

Device-side tricks:
 1. Everything under the max folds into ONE matmul per (b,i) row with an
    augmented contraction axis K = E + 1 + Z = 81:
      lhsT_aug[81,128] = [W_e ; -1e9*ones(1,H) ; W_j]          (constant)
      rhs_aug [81,Np]  = [e_feat[b,i,sel].T ; pad ; z[b,sel].T]  (streamed)
      PSUM[h,j] = ze + mask + zj   ->  reduce_max over j -> agg column
    zi + b_msg commute out of the max and fold into the final linear, whose
    z@Wu_z part is computed on the host (tiny, exact f32).
 2. The host compacts the j axis per row: only j with adj=1 participate in
    the max, so each row streams just its active columns (padded to the
    global max count Np, pad columns carry mask=1 -> -1e9).  This cuts PE,
    DVE and DMA work by ~ N/Np.
 3. The stream is plane-major [81, IH*Np] so one DMA per G-row block moves
    G*Np*2 contiguous bytes per partition, spread over all SDMA engines.
"""

import numpy as np
import ml_dtypes

import concourse.bacc as bacc
import concourse.mybir as mybir
import concourse.tile as tile
from concourse import bass_utils
from concourse.bass_interp import get_hw_module
from contextlib import ExitStack

B, N, Z, E, H = 4, 512, 64, 16, 128
NCORES = 8
IH = N * B // NCORES          # 256 destination rows per core
KAUG = E + 1 + Z              # 81
G = 32                        # rows per DMA block
NBLK = IH // G                # blocks per core
RG = 4                        # rows per grouped reduce (PSUM banks per tile)
BANK = 512                    # f32 elems per PSUM bank
# Of the G//RG row-groups per block, this many are reduced directly from
# PSUM by DVE; the rest are drained by ACT into bf16 SBUF and max-reduced
# by DVE via a 2-level tensor_tensor(max) tree in 2x mode.
DIRECT_PER_BLOCK = 1

F32 = mybir.dt.float32
BF16 = mybir.dt.bfloat16
NP_BF16 = ml_dtypes.bfloat16

TRACE = False                 # test.py sets True to capture an NTFF profile
TRACE_DIR = None              # optional fixed dir for trace artifacts
LAST_RESULTS = None           # BassKernelResults of the last run (for test.py)

_MODULE_CACHE = {}


def _ensure_ntff_hook():
    """The agent image's antenv lacks axon_hooks; recreate it so
    run_bass_kernel_spmd(trace=True) can reach the axon NTFF profiler."""
    import sys
    import types

    try:
        import antenv.axon_hooks  # noqa: F401

        return
    except ImportError:
        pass
    import antenv
    from trn_agent_boot.trn_boot import _ntff_profile_via_ctypes

    state = {"h": _ntff_profile_via_ctypes("/opt/axon/libaxon_pjrt.so")}
    mod = types.ModuleType("antenv.axon_hooks")
    mod.get_axon_ntff_profile_hook = lambda: state["h"]
    mod.set_axon_ntff_profile_hook = lambda h: state.__setitem__("h", h)
    sys.modules["antenv.axon_hooks"] = mod
    antenv.axon_hooks = mod


def _build_module(npad):
    nc = bacc.Bacc(
        "TRN2",
        target_bir_lowering=False,
        debug=False,
        enable_asserts=False,
        num_devices=NCORES,
    )

    stream = nc.dram_tensor("stream", [KAUG, IH * npad], BF16, kind="ExternalInput")
    lhst = nc.dram_tensor("lhst", [KAUG, H], BF16, kind="ExternalInput")
    zit = nc.dram_tensor("zit", [H, IH], F32, kind="ExternalInput")
    hostc = nc.dram_tensor("hostc", [H, IH], F32, kind="ExternalInput")
    wuh = nc.dram_tensor("wuh", [H, H], F32, kind="ExternalInput")
    ident = nc.dram_tensor("ident", [H, H], F32, kind="ExternalInput")
    out = nc.dram_tensor("out", [IH, H], F32, kind="ExternalOutput")

    with ExitStack() as ctx:
        tc = ctx.enter_context(tile.TileContext(nc))
        const = ctx.enter_context(tc.tile_pool(name="const", bufs=1))
        mega = ctx.enter_context(tc.tile_pool(name="mega", bufs=3))
        psum = ctx.enter_context(tc.tile_pool(name="psum", bufs=2, space="PSUM"))

        lhst_sb = const.tile([KAUG, H], BF16, tag="lhst")
        nc.scalar.dma_start(lhst_sb[:, :], lhst.ap())
        zit_sb = const.tile([H, IH], F32, tag="zit")
        nc.scalar.dma_start(zit_sb[:, :], zit.ap())
        hostc_sb = const.tile([H, IH], F32, tag="hostc")
        nc.scalar.dma_start(hostc_sb[:, :], hostc.ap())
        wuh_sb = const.tile([H, H], F32, tag="wuh")
        nc.scalar.dma_start(wuh_sb[:, :], wuh.ap())
        ident_sb = const.tile([H, H], F32, tag="ident")
        nc.scalar.dma_start(ident_sb[:, :], ident.ap())

        magg = const.tile([H, IH], F32, tag="magg")

        # npad is a multiple of 16, so two clean halvings are available.
        nh = npad // 2
        nq = npad // 4
        stage_pool = ctx.enter_context(tc.tile_pool(name="stage", bufs=5))

        # Ramp-up: small first blocks so the PE starts within ~1 us of launch
        # instead of waiting for a full 32-row block to land; bigger late
        # blocks for DMA packet efficiency.
        sizes = [4, 4, 8, 16, 32, 32, 32, 64, 64]
        assert sum(sizes) == IH

        stream_ap = stream.ap()
        row0 = 0
        for blk, gsz in enumerate(sizes):
            mb = mega.tile([KAUG, gsz * npad], BF16, tag="mega")
            nc.sync.dma_start(
                mb[:, :],
                stream_ap[:, row0 * npad : (row0 + gsz) * npad],
            )
            # direct groups spread evenly through the block
            ngrp = gsz // RG
            ndir = max(0, round(ngrp * DIRECT_PER_BLOCK / (G // RG)))
            for g4 in range(ngrp):
                ps = psum.tile([H, RG * BANK], F32, tag="ps")
                for r in range(RG):
                    g = g4 * RG + r
                    nc.tensor.matmul(
                        ps[:, r * BANK : r * BANK + npad],
                        lhst_sb[:, :],
                        mb[:, g * npad : (g + 1) * npad],
                        start=True,
                        stop=True,
                    )
                i0 = row0 + g4 * RG
                ps_rows = ps[:, :].rearrange("p (g j) -> p g j", g=RG)
                if g4 < ndir:
                    nc.vector.reduce_max(
                        magg[:, i0 : i0 + RG],
                        ps_rows[:, :, :npad],
                        axis=mybir.AxisListType.X,
                    )
                else:
                    stage = stage_pool.tile([H, RG * npad], BF16, tag="stage")
                    st_rows = stage[:, :].rearrange("p (g j) -> p g j", g=RG)
                    nc.scalar.copy(st_rows[:, :, :], ps_rows[:, :, :npad])
                    half = stage_pool.tile([H, RG * nh], BF16, tag="half")
                    hf_rows = half[:, :].rearrange("p (g j) -> p g j", g=RG)
                    nc.vector.tensor_tensor(
                        hf_rows[:, :, :],
                        st_rows[:, :, :nh],
                        st_rows[:, :, nh:npad],
                        mybir.AluOpType.max,
                    )
                    quar = stage_pool.tile([H, RG * nq], BF16, tag="quar")
                    qr_rows = quar[:, :].rearrange("p (g j) -> p g j", g=RG)
                    nc.vector.tensor_tensor(
                        qr_rows[:, :, :],
                        hf_rows[:, :, :nq],
                        hf_rows[:, :, nq:nh],
                        mybir.AluOpType.max,
                    )
                    nc.vector.reduce_max(
                        magg[:, i0 : i0 + RG],
                        qr_rows[:, :, :],
                        axis=mybir.AxisListType.X,
                    )
            row0 += gsz

            # Emit the output tail for each finished half so it overlaps the
            # remaining loop instead of serializing at the end.
            for t in range(IH // H):
                if row0 != (t + 1) * H:
                    continue
                cols = slice(t * H, (t + 1) * H)
                aggt = const.tile([H, H], F32, tag=f"aggt{t}")
                nc.vector.tensor_add(aggt[:, :], magg[:, cols], zit_sb[:, cols])
                psf = psum.tile([H, RG * BANK], F32, tag="ps")
                nc.tensor.matmul(
                    psf[:, :H], wuh_sb[:, :], aggt[:, :], start=True, stop=True
                )
                outt = const.tile([H, H], F32, tag=f"outt{t}")
                nc.vector.tensor_add(outt[:, :], psf[:, :H], hostc_sb[:, cols])
                pst = psum.tile([H, RG * BANK], F32, tag="ps")
                nc.tensor.transpose(pst[:, :H], outt[:, :], ident_sb[:, :])
                osb = const.tile([H, H], F32, tag=f"osb{t}")
                nc.scalar.copy(osb[:, :], pst[:, :H])
                nc.sync.dma_start(out.ap()[cols, :], osb[:, :])

    nc.compile()
    nc.m = get_hw_module(nc.m)
    return nc


def _prepare(z, e_feat, adj, W_msg, b_msg, W_upd, b_upd):
    """Host-side sharding + compaction.  Returns (in_maps, npad)."""
    W_i, W_j, W_e = W_msg[:Z], W_msg[Z : 2 * Z], W_msg[2 * Z :]
    Wu_z, Wu_h = W_upd[:Z], W_upd[Z:]

    counts = (adj > 0).sum(axis=-1)
    npad = int(counts.max())
    npad = max(16, (npad + 15) // 16 * 16)
    npad = min(npad, N)

    lhst_np = np.concatenate(
        [W_e, np.full((1, H), -1e9, np.float32), W_j], axis=0
    ).astype(NP_BF16)
    wuh_np = np.ascontiguousarray(Wu_h, np.float32)
    ident_np = np.eye(H, dtype=np.float32)

    in_maps = []
    for c in range(NCORES):
        b, half = divmod(c, NCORES // B)
        sl = slice(half * IH, (half + 1) * IH)
        adj_blk = adj[b, sl] > 0                      # [IH, N] bool
        # active columns first (stable -> ascending j), inactive fill the pad
        order = np.argsort(~adj_blk, axis=-1, kind="stable")[:, :npad]
        e_sel = np.take_along_axis(
            e_feat[b, sl], order[:, :, None], axis=1
        )                                             # [IH, npad, E]
        z_sel = z[b][order]                           # [IH, npad, Z]
        msk = ~np.take_along_axis(adj_blk, order, axis=1)  # True on pad cols

        stream = np.empty((KAUG, IH, npad), dtype=NP_BF16)
        stream[:E] = e_sel.transpose(2, 0, 1)
        stream[E] = msk
        stream[E + 1 :] = z_sel.transpose(2, 0, 1)

        in_maps.append(
            {
                "stream": stream.reshape(KAUG, IH * npad),
                "lhst": lhst_np,
                "zit": np.ascontiguousarray(
                    (z[b, sl] @ W_i).T + b_msg[:, None], dtype=np.float32
                ),
                "hostc": np.ascontiguousarray(
                    (z[b, sl] @ Wu_z + b_upd).T, dtype=np.float32
                ),
                "wuh": wuh_np,
                "ident": ident_np,
            }
        )
    return in_maps, npad


def kernel(z, e_feat, adj, W_msg, b_msg, W_upd, b_upd):
    global LAST_RESULTS

    z = np.asarray(z, np.float32)
    e_feat = np.asarray(e_feat, np.float32)
    adj = np.asarray(adj)
    W_msg = np.asarray(W_msg, np.float32)
    b_msg = np.asarray(b_msg, np.float32)
    W_upd = np.asarray(W_upd, np.float32)
    b_upd = np.asarray(b_upd, np.float32)

    in_maps, npad = _prepare(z, e_feat, adj, W_msg, b_msg, W_upd, b_upd)

    if npad not in _MODULE_CACHE:
        _MODULE_CACHE[npad] = _build_module(npad)
    nc = _MODULE_CACHE[npad]

    if TRACE:
        _ensure_ntff_hook()
    res = bass_utils.run_bass_kernel_spmd(
        nc, in_maps, core_ids=list(range(NCORES)), trace=TRACE, tmpdir=TRACE_DIR
    )
    LAST_RESULTS = res

    full = np.empty((B, N, H), np.float32)
    for c in range(NCORES):
        b, half = divmod(c, NCORES // B)
        full[b, half * IH : (half + 1) * IH] = res.results[c]["out"]
    return full


if __name__ == "__main__":
    rng = np.random.default_rng(0)
    ins = {
        "z": rng.standard_normal((B, N, Z)).astype(np.float32),
        "e_feat": rng.standard_normal((B, N, N, E)).astype(np.float32),
        "adj": (rng.random((B, N, N)) < 0.5).astype(np.int32),
        "W_msg": (rng.standard_normal((2 * Z + E, H)) * 0.1).astype(np.float32),
        "b_msg": np.zeros(H, np.float32),
        "W_upd": (rng.standard_normal((Z + H, H)) * 0.1).astype(np.float32),
        "b_upd": np.zeros(H, np.float32),
    }
    out = kernel(**ins)
    print("out", out.shape, out.dtype, float(np.abs(out).max()))


# revision 35
# speedup vs baseline: 1.0072x; 1.0072x over previous
"""Trainium2 Bass kernel for nn_MultiMPNN (gnn_message_passing).

Reference computation (B=4, N=512, Z=64, E=16, H=128):
    msgs[b,i,j,:] = z[b,i]@W_i + z[b,j]@W_j + e_feat[b,i,j]@W_e + b_msg
    agg[b,i,:]    = max_j (msgs + (adj>0 ? 0 : -inf))
    out           = z@Wu_z + agg@Wu_h + b_upd

Sharding: 8 cores = (batch b, half of destination rows i).  Each core owns
256 i-rows and the full j axis.

Device-side tricks:
 1. Everything under the max folds into ONE matmul per (b,i) row with an
    augmented contraction axis K = E + 1 + Z = 81:
      lhsT_aug[81,128] = [W_e ; -1e9*ones(1,H) ; W_j]          (constant)
      rhs_aug [81,Np]  = [e_feat[b,i,sel].T ; pad ; z[b,sel].T]  (streamed)
      PSUM[h,j] = ze + mask + zj   ->  reduce_max over j -> agg column
    zi + b_msg commute out of the max and fold into the final linear, whose
    z@Wu_z part is computed on the host (tiny, exact f32).
 2. The host compacts the j axis per row: only j with adj=1 participate in
    the max, so each row streams just its active columns (padded to the
    global max count Np, pad columns carry mask=1 -> -1e9).  This cuts PE,
    DVE and DMA work by ~ N/Np.
 3. The stream is plane-major [81, IH*Np] so one DMA per G-row block moves
    G*Np*2 contiguous bytes per partition, spread over all SDMA engines.
"""

import numpy as np
import ml_dtypes

import concourse.bacc as bacc
import concourse.mybir as mybir
import concourse.tile as tile
from concourse import bass_utils
from concourse.bass_interp import get_hw_module
from contextlib import ExitStack

B, N, Z, E, H = 4, 512, 64, 16, 128
NCORES = 8
IH = N * B // NCORES          # 256 destination rows per core
KAUG = E + 1 + Z              # 81
G = 32                        # rows per DMA block
NBLK = IH // G                # blocks per core
RG = 4                        # rows per grouped reduce (PSUM banks per tile)
BANK = 512                    # f32 elems per PSUM bank
# Of the G//RG row-groups per block, this many are reduced directly from
# PSUM by DVE; the rest are drained by ACT into bf16 SBUF and max-reduced
# by DVE via a 2-level tensor_tensor(max) tree in 2x mode.
DIRECT_PER_BLOCK = 1

F32 = mybir.dt.float32
BF16 = mybir.dt.bfloat16
NP_BF16 = ml_dtypes.bfloat16

TRACE = False                 # test.py sets True to capture an NTFF profile
TRACE_DIR = None              # optional fixed dir for trace artifacts
LAST_RESULTS = None           # BassKernelResults of the last run (for test.py)

_MODULE_CACHE = {}


def _ensure_ntff_hook():
    """The agent image's antenv lacks axon_hooks; recreate it so
    run_bass_kernel_spmd(trace=True) can reach the axon NTFF profiler."""
    import sys
    import types

    try:
        import antenv.axon_hooks  # noqa: F401

        return
    except ImportError:
        pass
    import antenv
    from trn_agent_boot.trn_boot import _ntff_profile_via_ctypes

    state = {"h": _ntff_profile_via_ctypes("/opt/axon/libaxon_pjrt.so")}
    mod = types.ModuleType("antenv.axon_hooks")
    mod.get_axon_ntff_profile_hook = lambda: state["h"]
    mod.set_axon_ntff_profile_hook = lambda h: state.__setitem__("h", h)
    sys.modules["antenv.axon_hooks"] = mod
    antenv.axon_hooks = mod


def _build_module(npad):
    nc = bacc.Bacc(
        "TRN2",
        target_bir_lowering=False,
        debug=False,
        enable_asserts=False,
        num_devices=NCORES,
    )

    stream = nc.dram_tensor("stream", [KAUG, IH * npad], BF16, kind="ExternalInput")
    lhst = nc.dram_tensor("lhst", [KAUG, H], BF16, kind="ExternalInput")
    zit = nc.dram_tensor("zit", [H, IH], F32, kind="ExternalInput")
    hostc = nc.dram_tensor("hostc", [H, IH], F32, kind="ExternalInput")
    wuh = nc.dram_tensor("wuh", [H, H], F32, kind="ExternalInput")
    ident = nc.dram_tensor("ident", [H, H], F32, kind="ExternalInput")
    out = nc.dram_tensor("out", [IH, H], F32, kind="ExternalOutput")

    with ExitStack() as ctx:
        tc = ctx.enter_context(tile.TileContext(nc))
        const = ctx.enter_context(tc.tile_pool(name="const", bufs=1))
        mega = ctx.enter_context(tc.tile_pool(name="mega", bufs=3))
        psum = ctx.enter_context(tc.tile_pool(name="psum", bufs=2, space="PSUM"))

        lhst_sb = const.tile([KAUG, H], BF16, tag="lhst")
        nc.scalar.dma_start(lhst_sb[:, :], lhst.ap())
        zit_sb = const.tile([H, IH], F32, tag="zit")
        nc.scalar.dma_start(zit_sb[:, :], zit.ap())
        hostc_sb = const.tile([H, IH], F32, tag="hostc")
        nc.scalar.dma_start(hostc_sb[:, :], hostc.ap())
        wuh_sb = const.tile([H, H], F32, tag="wuh")
        nc.scalar.dma_start(wuh_sb[:, :], wuh.ap())
        ident_sb = const.tile([H, H], F32, tag="ident")
        nc.scalar.dma_start(ident_sb[:, :], ident.ap())

        magg = const.tile([H, IH], F32, tag="magg")

        # npad is a multiple of 16, so two clean halvings are available.
        nh = npad // 2
        nq = npad // 4
        stage_pool = ctx.enter_context(tc.tile_pool(name="stage", bufs=5))

        # Ramp-up: small first blocks so the PE starts within ~1 us of launch
        # instead of waiting for a full 32-row block to land; bigger late
        # blocks for DMA packet efficiency.
        sizes = [4, 4, 8, 16] + [G] * 7
        assert sum(sizes) == IH

        stream_ap = stream.ap()
        row0 = 0
        for blk, gsz in enumerate(sizes):
            mb = mega.tile([KAUG, gsz * npad], BF16, tag="mega")
            nc.sync.dma_start(
                mb[:, :],
                stream_ap[:, row0 * npad : (row0 + gsz) * npad],
            )
            # direct groups spread evenly through the block
            ngrp = gsz // RG
            ndir = max(0, round(ngrp * DIRECT_PER_BLOCK / (G // RG)))
            for g4 in range(ngrp):
                ps = psum.tile([H, RG * BANK], F32, tag="ps")
                for r in range(RG):
                    g = g4 * RG + r
                    nc.tensor.matmul(
                        ps[:, r * BANK : r * BANK + npad],
                        lhst_sb[:, :],
                        mb[:, g * npad : (g + 1) * npad],
                        start=True,
                        stop=True,
                    )
                i0 = row0 + g4 * RG
                ps_rows = ps[:, :].rearrange("p (g j) -> p g j", g=RG)
                if g4 < ndir:
                    nc.vector.reduce_max(
                        magg[:, i0 : i0 + RG],
                        ps_rows[:, :, :npad],
                        axis=mybir.AxisListType.X,
                    )
                else:
                    stage = stage_pool.tile([H, RG * npad], BF16, tag="stage")
                    st_rows = stage[:, :].rearrange("p (g j) -> p g j", g=RG)
                    nc.scalar.copy(st_rows[:, :, :], ps_rows[:, :, :npad])
                    half = stage_pool.tile([H, RG * nh], BF16, tag="half")
                    hf_rows = half[:, :].rearrange("p (g j) -> p g j", g=RG)
                    nc.vector.tensor_tensor(
                        hf_rows[:, :, :],
                        st_rows[:, :, :nh],
                        st_rows[:, :, nh:npad],
                        mybir.AluOpType.max,
                    )
                    quar = stage_pool.tile([H, RG * nq], BF16, tag="quar")
                    qr_rows = quar[:, :].rearrange("p (g j) -> p g j", g=RG)
                    nc.vector.tensor_tensor(
                        qr_rows[:, :, :],
                        hf_rows[:, :, :nq],
                        hf_rows[:, :, nq:nh],
                        mybir.AluOpType.max,
                    )
                    nc.vector.reduce_max(
                        magg[:, i0 : i0 + RG],
                        qr_rows[:, :, :],
                        axis=mybir.AxisListType.X,
                    )
            row0 += gsz

            # Emit the output tail for each finished half so it overlaps the
            # remaining loop instead of serializing at the end.
            for t in range(IH // H):
                if row0 != (t + 1) * H:
                    continue
                cols = slice(t * H, (t + 1) * H)
                aggt = const.tile([H, H], F32, tag=f"aggt{t}")
                nc.vector.tensor_add(aggt[:, :], magg[:, cols], zit_sb[:, cols])
                psf = psum.tile([H, RG * BANK], F32, tag="ps")
                nc.tensor.matmul(
                    psf[:, :H], wuh_sb[:, :], aggt[:, :], start=True, stop=True
                )
                outt = const.tile([H, H], F32, tag=f"outt{t}")
                nc.vector.tensor_add(outt[:, :], psf[:, :H], hostc_sb[:, cols])
                pst = psum.tile([H, RG * BANK], F32, tag="ps")
                nc.tensor.transpose(pst[:, :H], outt[:, :], ident_sb[:, :])
                osb = const.tile([H, H], F32, tag=f"osb{t}")
                nc.scalar.copy(osb[:, :], pst[:, :H])
                nc.sync.dma_start(out.ap()[cols, :], osb[:, :])

    nc.compile()
    nc.m = get_hw_module(nc.m)
    return nc


def _prepare(z, e_feat, adj, W_msg, b_msg, W_upd, b_upd):
    """Host-side sharding + compaction.  Returns (in_maps, npad)."""
    W_i, W_j, W_e = W_msg[:Z], W_msg[Z : 2 * Z], W_msg[2 * Z :]
    Wu_z, Wu_h = W_upd[:Z], W_upd[Z:]

    counts = (adj > 0).sum(axis=-1)
    npad = int(counts.max())
    npad = max(16, (npad + 15) // 16 * 16)
    npad = min(npad, N)

    lhst_np = np.concatenate(
        [W_e, np.full((1, H), -1e9, np.float32), W_j], axis=0
    ).astype(NP_BF16)
    wuh_np = np.ascontiguousarray(Wu_h, np.float32)
    ident_np = np.eye(H, dtype=np.float32)

    in_maps = []
    for c in range(NCORES):
        b, half = divmod(c, NCORES // B)
        sl = slice(half * IH, (half + 1) * IH)
        adj_blk = adj[b, sl] > 0                      # [IH, N] bool
        # active columns first (stable -> ascending j), inactive fill the pad
        order = np.argsort(~adj_blk, axis=-1, kind="stable")[:, :npad]
        e_sel = np.take_along_axis(
            e_feat[b, sl], order[:, :, None], axis=1
        )                                             # [IH, npad, E]
        z_sel = z[b][order]                           # [IH, npad, Z]
        msk = ~np.take_along_axis(adj_blk, order, axis=1)  # True on pad cols

        stream = np.empty((KAUG, IH, npad), dtype=NP_BF16)
        stream[:E] = e_sel.transpose(2, 0, 1)
        stream[E] = msk
        stream[E + 1 :] = z_sel.transpose(2, 0, 1)

        in_maps.append(
            {
                "stream": stream.reshape(KAUG, IH * npad),
                "lhst": lhst_np,
                "zit": np.ascontiguousarray(
                    (z[b, sl] @ W_i).T + b_msg[:, None], dtype=np.float32
                ),
                "hostc": np.ascontiguousarray(
                    (z[b, sl] @ Wu_z + b_upd).T, dtype=np.float32
                ),
                "wuh": wuh_np,
                "ident": ident_np,
            }
        )
    return in_maps, npad


def kernel(z, e_feat, adj, W_msg, b_msg, W_upd, b_upd):
    global LAST_RESULTS

    z = np.asarray(z, np.float32)
    e_feat = np.asarray(e_feat, np.float32)
    adj = np.asarray(adj)
    W_msg = np.asarray(W_msg, np.float32)
    b_msg = np.asarray(b_msg, np.float32)
    W_upd = np.asarray(W_upd, np.float32)
    b_upd = np.asarray(b_upd, np.float32)

    in_maps, npad = _prepare(z, e_feat, adj, W_msg, b_msg, W_upd, b_upd)

    if npad not in _MODULE_CACHE:
        _MODULE_CACHE[npad] = _build_module(npad)
    nc = _MODULE_CACHE[npad]

    if TRACE:
        _ensure_ntff_hook()
    res = bass_utils.run_bass_kernel_spmd(
        nc, in_maps, core_ids=list(range(NCORES)), trace=TRACE, tmpdir=TRACE_DIR
    )
    LAST_RESULTS = res

    full = np.empty((B, N, H), np.float32)
    for c in range(NCORES):
        b, half = divmod(c, NCORES // B)
        full[b, half * IH : (half + 1) * IH] = res.results[c]["out"]
    return full


if __name__ == "__main__":
    rng = np.random.default_rng(0)
    ins = {
        "z": rng.standard_normal((B, N, Z)).astype(np.float32),
        "e_feat": rng.standard_normal((B, N, N, E)).astype(np.float32),
        "adj": (rng.random((B, N, N)) < 0.5).astype(np.int32),
        "W_msg": (rng.standard_normal((2 * Z + E, H)) * 0.1).astype(np.float32),
        "b_msg": np.zeros(H, np.float32),
        "W_upd": (rng.standard_normal((Z + H, H)) * 0.1).astype(np.float32),
        "b_upd": np.zeros(H, np.float32),
    }
    out = kernel(**ins)
    print("out", out.shape, out.dtype, float(np.abs(out).max()))


# revision 37
# speedup vs baseline: 1.0621x; 1.0546x over previous
"""Trainium2 Bass kernel for nn_MultiMPNN (gnn_message_passing).

Reference computation (B=4, N=512, Z=64, E=16, H=128):
    msgs[b,i,j,:] = z[b,i]@W_i + z[b,j]@W_j + e_feat[b,i,j]@W_e + b_msg
    agg[b,i,:]    = max_j (msgs + (adj>0 ? 0 : -inf))
    out           = z@Wu_z + agg@Wu_h + b_upd

Sharding: 8 cores = (batch b, half of destination rows i).  Each core owns
256 i-rows and the full j axis.

Device-side tricks:
 1. Everything under the max folds into ONE matmul per (b,i) row with an
    augmented contraction axis K = E + 1 + Z = 81:
      lhsT_aug[81,128] = [W_e ; -1e9*ones(1,H) ; W_j]          (constant)
      rhs_aug [81,Np]  = [e_feat[b,i,sel].T ; pad ; z[b,sel].T]  (streamed)
      PSUM[h,j] = ze + mask + zj   ->  reduce_max over j -> agg column
    zi + b_msg commute out of the max and fold into the final linear, whose
    z@Wu_z part is computed on the host (tiny, exact f32).
 2. The host compacts the j axis per row: only j with adj=1 participate in
    the max, so each row streams just its active columns (padded to the
    global max count Np, pad columns carry mask=1 -> -1e9).  This cuts PE,
    DVE and DMA work by ~ N/Np.
 3. The stream is plane-major [81, IH*Np] so one DMA per G-row block moves
    G*Np*2 contiguous bytes per partition, spread over all SDMA engines.
"""

import numpy as np
import ml_dtypes

import concourse.bacc as bacc
import concourse.mybir as mybir
import concourse.tile as tile
from concourse import bass_utils
from concourse.bass_interp import get_hw_module
from contextlib import ExitStack

B, N, Z, E, H = 4, 512, 64, 16, 128
NCORES = 8
IH = N * B // NCORES          # 256 destination rows per core
KAUG = E + 1 + Z              # 81
G = 32                        # rows per DMA block
NBLK = IH // G                # blocks per core
RG = 2                        # rows per grouped reduce (PSUM banks per tile)
BANK = 512                    # f32 elems per PSUM bank
# Of the G//RG row-groups per block, this many are reduced directly from
# PSUM by DVE; the rest are drained by ACT into bf16 SBUF and max-reduced
# by DVE via a 2-level tensor_tensor(max) tree in 2x mode.
DIRECT_PER_BLOCK = 2

F32 = mybir.dt.float32
BF16 = mybir.dt.bfloat16
NP_BF16 = ml_dtypes.bfloat16

TRACE = False                 # test.py sets True to capture an NTFF profile
TRACE_DIR = None              # optional fixed dir for trace artifacts
LAST_RESULTS = None           # BassKernelResults of the last run (for test.py)

_MODULE_CACHE = {}


def _ensure_ntff_hook():
    """The agent image's antenv lacks axon_hooks; recreate it so
    run_bass_kernel_spmd(trace=True) can reach the axon NTFF profiler."""
    import sys
    import types

    try:
        import antenv.axon_hooks  # noqa: F401

        return
    except ImportError:
        pass
    import antenv
    from trn_agent_boot.trn_boot import _ntff_profile_via_ctypes

    state = {"h": _ntff_profile_via_ctypes("/opt/axon/libaxon_pjrt.so")}
    mod = types.ModuleType("antenv.axon_hooks")
    mod.get_axon_ntff_profile_hook = lambda: state["h"]
    mod.set_axon_ntff_profile_hook = lambda h: state.__setitem__("h", h)
    sys.modules["antenv.axon_hooks"] = mod
    antenv.axon_hooks = mod


def _build_module(npad):
    nc = bacc.Bacc(
        "TRN2",
        target_bir_lowering=False,
        debug=False,
        enable_asserts=False,
        num_devices=NCORES,
    )

    stream = nc.dram_tensor("stream", [KAUG, IH * npad], BF16, kind="ExternalInput")
    lhst = nc.dram_tensor("lhst", [KAUG, H], BF16, kind="ExternalInput")
    zit = nc.dram_tensor("zit", [H, IH], F32, kind="ExternalInput")
    hostc = nc.dram_tensor("hostc", [H, IH], F32, kind="ExternalInput")
    wuh = nc.dram_tensor("wuh", [H, H], F32, kind="ExternalInput")
    ident = nc.dram_tensor("ident", [H, H], F32, kind="ExternalInput")
    out = nc.dram_tensor("out", [IH, H], F32, kind="ExternalOutput")

    with ExitStack() as ctx:
        tc = ctx.enter_context(tile.TileContext(nc))
        const = ctx.enter_context(tc.tile_pool(name="const", bufs=1))
        mega = ctx.enter_context(tc.tile_pool(name="mega", bufs=3))
        psum = ctx.enter_context(tc.tile_pool(name="psum", bufs=4, space="PSUM"))

        lhst_sb = const.tile([KAUG, H], BF16, tag="lhst")
        nc.scalar.dma_start(lhst_sb[:, :], lhst.ap())
        zit_sb = const.tile([H, IH], F32, tag="zit")
        nc.scalar.dma_start(zit_sb[:, :], zit.ap())
        hostc_sb = const.tile([H, IH], F32, tag="hostc")
        nc.scalar.dma_start(hostc_sb[:, :], hostc.ap())
        wuh_sb = const.tile([H, H], F32, tag="wuh")
        nc.scalar.dma_start(wuh_sb[:, :], wuh.ap())
        ident_sb = const.tile([H, H], F32, tag="ident")
        nc.scalar.dma_start(ident_sb[:, :], ident.ap())

        magg = const.tile([H, IH], F32, tag="magg")

        # npad is a multiple of 16, so two clean halvings are available.
        nh = npad // 2
        nq = npad // 4
        stage_pool = ctx.enter_context(tc.tile_pool(name="stage", bufs=5))

        # Ramp-up: small first blocks so the PE starts within ~1 us of launch
        # instead of waiting for a full 32-row block to land; bigger late
        # blocks for DMA packet efficiency.
        sizes = [4, 4, 8, 16] + [G] * 7
        assert sum(sizes) == IH

        stream_ap = stream.ap()
        row0 = 0
        for blk, gsz in enumerate(sizes):
            mb = mega.tile([KAUG, gsz * npad], BF16, tag="mega")
            nc.sync.dma_start(
                mb[:, :],
                stream_ap[:, row0 * npad : (row0 + gsz) * npad],
            )
            # direct groups spread evenly through the block
            ngrp = gsz // RG
            ndir = max(0, round(ngrp * DIRECT_PER_BLOCK / (G // RG)))
            for g4 in range(ngrp):
                ps = psum.tile([H, RG * BANK], F32, tag="ps")
                for r in range(RG):
                    g = g4 * RG + r
                    nc.tensor.matmul(
                        ps[:, r * BANK : r * BANK + npad],
                        lhst_sb[:, :],
                        mb[:, g * npad : (g + 1) * npad],
                        start=True,
                        stop=True,
                    )
                i0 = row0 + g4 * RG
                ps_rows = ps[:, :].rearrange("p (g j) -> p g j", g=RG)
                if g4 < ndir:
                    nc.vector.reduce_max(
                        magg[:, i0 : i0 + RG],
                        ps_rows[:, :, :npad],
                        axis=mybir.AxisListType.X,
                    )
                else:
                    stage = stage_pool.tile([H, RG * npad], BF16, tag="stage")
                    st_rows = stage[:, :].rearrange("p (g j) -> p g j", g=RG)
                    nc.scalar.copy(st_rows[:, :, :], ps_rows[:, :, :npad])
                    half = stage_pool.tile([H, RG * nh], BF16, tag="half")
                    hf_rows = half[:, :].rearrange("p (g j) -> p g j", g=RG)
                    nc.vector.tensor_tensor(
                        hf_rows[:, :, :],
                        st_rows[:, :, :nh],
                        st_rows[:, :, nh:npad],
                        mybir.AluOpType.max,
                    )
                    quar = stage_pool.tile([H, RG * nq], BF16, tag="quar")
                    qr_rows = quar[:, :].rearrange("p (g j) -> p g j", g=RG)
                    nc.vector.tensor_tensor(
                        qr_rows[:, :, :],
                        hf_rows[:, :, :nq],
                        hf_rows[:, :, nq:nh],
                        mybir.AluOpType.max,
                    )
                    nc.vector.reduce_max(
                        magg[:, i0 : i0 + RG],
                        qr_rows[:, :, :],
                        axis=mybir.AxisListType.X,
                    )
            row0 += gsz

        aggt = const.tile([H, IH], F32, tag="aggt")
        nc.vector.tensor_add(aggt[:, :], magg[:, :], zit_sb[:, :])

        psf = psum.tile([H, RG * BANK], F32, tag="ps")
        nc.tensor.matmul(psf[:, :IH], wuh_sb[:, :], aggt[:, :], start=True, stop=True)

        outt = const.tile([H, IH], F32, tag="outt")
        nc.vector.tensor_add(outt[:, :], psf[:, :IH], hostc_sb[:, :])

        out_ap = out.ap()
        for t in range(IH // H):
            pst = psum.tile([H, RG * BANK], F32, tag="ps")
            nc.tensor.transpose(
                pst[:, :H], outt[:, t * H : (t + 1) * H], ident_sb[:, :]
            )
            osb = const.tile([H, H], F32, tag=f"osb{t}")
            nc.scalar.copy(osb[:, :], pst[:, :H])
            nc.sync.dma_start(out_ap[t * H : (t + 1) * H, :], osb[:, :])

    nc.compile()
    nc.m = get_hw_module(nc.m)
    return nc


def _prepare(z, e_feat, adj, W_msg, b_msg, W_upd, b_upd):
    """Host-side sharding + compaction.  Returns (in_maps, npad)."""
    W_i, W_j, W_e = W_msg[:Z], W_msg[Z : 2 * Z], W_msg[2 * Z :]
    Wu_z, Wu_h = W_upd[:Z], W_upd[Z:]

    counts = (adj > 0).sum(axis=-1)
    npad = int(counts.max())
    npad = max(16, (npad + 15) // 16 * 16)
    npad = min(npad, N)

    lhst_np = np.concatenate(
        [W_e, np.full((1, H), -1e9, np.float32), W_j], axis=0
    ).astype(NP_BF16)
    wuh_np = np.ascontiguousarray(Wu_h, np.float32)
    ident_np = np.eye(H, dtype=np.float32)

    in_maps = []
    for c in range(NCORES):
        b, half = divmod(c, NCORES // B)
        sl = slice(half * IH, (half + 1) * IH)
        adj_blk = adj[b, sl] > 0                      # [IH, N] bool
        # active columns first (stable -> ascending j), inactive fill the pad
        order = np.argsort(~adj_blk, axis=-1, kind="stable")[:, :npad]
        e_sel = np.take_along_axis(
            e_feat[b, sl], order[:, :, None], axis=1
        )                                             # [IH, npad, E]
        z_sel = z[b][order]                           # [IH, npad, Z]
        msk = ~np.take_along_axis(adj_blk, order, axis=1)  # True on pad cols

        stream = np.empty((KAUG, IH, npad), dtype=NP_BF16)
        stream[:E] = e_sel.transpose(2, 0, 1)
        stream[E] = msk
        stream[E + 1 :] = z_sel.transpose(2, 0, 1)

        in_maps.append(
            {
                "stream": stream.reshape(KAUG, IH * npad),
                "lhst": lhst_np,
                "zit": np.ascontiguousarray(
                    (z[b, sl] @ W_i).T + b_msg[:, None], dtype=np.float32
                ),
                "hostc": np.ascontiguousarray(
                    (z[b, sl] @ Wu_z + b_upd).T, dtype=np.float32
                ),
                "wuh": wuh_np,
                "ident": ident_np,
            }
        )
    return in_maps, npad


def kernel(z, e_feat, adj, W_msg, b_msg, W_upd, b_upd):
    global LAST_RESULTS

    z = np.asarray(z, np.float32)
    e_feat = np.asarray(e_feat, np.float32)
    adj = np.asarray(adj)
    W_msg = np.asarray(W_msg, np.float32)
    b_msg = np.asarray(b_msg, np.float32)
    W_upd = np.asarray(W_upd, np.float32)
    b_upd = np.asarray(b_upd, np.float32)

    in_maps, npad = _prepare(z, e_feat, adj, W_msg, b_msg, W_upd, b_upd)

    if npad not in _MODULE_CACHE:
        _MODULE_CACHE[npad] = _build_module(npad)
    nc = _MODULE_CACHE[npad]

    if TRACE:
        _ensure_ntff_hook()
    res = bass_utils.run_bass_kernel_spmd(
        nc, in_maps, core_ids=list(range(NCORES)), trace=TRACE, tmpdir=TRACE_DIR
    )
    LAST_RESULTS = res

    full = np.empty((B, N, H), np.float32)
    for c in range(NCORES):
        b, half = divmod(c, NCORES // B)
        full[b, half * IH : (half + 1) * IH] = res.results[c]["out"]
    return full


if __name__ == "__main__":
    rng = np.random.default_rng(0)
    ins = {
        "z": rng.standard_normal((B, N, Z)).astype(np.float32),
        "e_feat": rng.standard_normal((B, N, N, E)).astype(np.float32),
        "adj": (rng.random((B, N, N)) < 0.5).astype(np.int32),
        "W_msg": (rng.standard_normal((2 * Z + E, H)) * 0.1).astype(np.float32),
        "b_msg": np.zeros(H, np.float32),
        "W_upd": (rng.standard_normal((Z + H, H)) * 0.1).astype(np.float32),
        "b_upd": np.zeros(H, np.float32),
    }
    out = kernel(**ins)
    print("out", out.shape, out.dtype, float(np.abs(out).max()))
